# revision 1
# baseline (speedup 1.0000x reference)
"""DeepSeek-MoE FFN (8 routed experts, top-2, SwiGLU, shared expert) on 8
Trainium2 NeuronCores.

Strategy: token-parallel. Each core takes N/8 = 2048 tokens and computes the
full mixture for them (all 8 routed experts densely, weighted by the dense
combine matrix, plus the shared expert); no collectives. Routing (gate
logits, top-2, softmax) runs in fp32 on device; expert matmuls run in bf16
with fp32 PSUM accumulation.

Per-core layouts (host-prepped, d-chunked so every DMA line is contiguous):
  xt   [128, 8, 2048] f32   xt[p, c, t]  = x[t, c*128+p]      (gate matmul)
  xtb  [128, 8, 2048] bf16  same, bf16                        (expert matmuls)
  gt   [128, 8, 8]    f32   gt[p, c, e]  = gate_w[e, c*128+p]
  wg   [9, 12, 128, 8, 128] bf16  wg[u, fc, p, c, f] = Wg_u[fc*128+f, c*128+p]
  wu   same layout for the up projection
  wd   [9, 12, 128, 1024]   bf16  wd[u, fc, p, d]    = Wd_u[d, fc*128+p]
  (unit 8 is the shared expert; its combine weight is fixed at 1.0)
  out  [128, 8, 2048] f32   out[p, c, t] = y[t, c*128+p]
"""

import sys

if '/opt/trn_rl_repo' not in sys.path:
    sys.path.insert(0, '/opt/trn_rl_repo')

from contextlib import ExitStack

import numpy as np
import ml_dtypes

import concourse.bass as bass
import concourse.tile as tile
import concourse.mybir as mybir
from concourse.alu_op_type import AluOpType
from concourse.vector_clock import ScopedClock

bf16 = ml_dtypes.bfloat16
F32 = mybir.dt.float32
BF = mybir.dt.bfloat16
AF = mybir.ActivationFunctionType
AX = mybir.AxisListType

# ---------------------------------------------------------------------------
# TileContext tail-drain fix: the stock exit emits one Drain carrying a sem
# wait per live logical proc, but walrus only accepts a single sync wait per
# SP instruction. Split the waits across preceding sync nops.
_MAX_WAITS = 1


def _patched_drain_and_barrier(self, tick_clock, wait_clock):
    nc = self.nc
    probe = nc.sync.nop()
    wait_clock.add_sem_waits(probe.ins, ScopedClock({None: tick_clock.global_clock}))
    si = probe.ins.sync_info
    waits = list(si.on_wait) if si is not None else []
    if len(waits) > _MAX_WAITS:
        probe.ins.sync_info = mybir.SyncInfo(on_wait=waits[:_MAX_WAITS], on_update=[])
        for k in range(_MAX_WAITS, len(waits), _MAX_WAITS):
            n = nc.sync.nop()
            n.ins.sync_info = mybir.SyncInfo(
                on_wait=waits[k:k + _MAX_WAITS], on_update=[]
            )
    nc.sync.drain()
    nc.all_engine_barrier()
    assert self.sems is not None
    popped = nc._tile_sem_poison_stack.pop()
    assert popped is self._sem_poison
    nc.clear_and_free_semaphores(list(self.sems.allocated().values()))
    nc.all_engine_barrier()


tile.TileContext._drain_and_barrier = _patched_drain_and_barrier

# ---------------------------------------------------------------------------
# This walrus build accepts only ONE sync wait per instruction. Hoist extra
# waits onto standalone same-engine NoOps placed immediately before.
_WSPLIT_ID = [0]


def _split_multi_waits(nc):
    for f in nc.m.functions:
        for bb in f.blocks:
            out = []
            changed = False
            for inst in bb.instructions:
                si = getattr(inst, 'sync_info', None)
                if si is not None and si.on_wait and len(si.on_wait) > 1:
                    changed = True
                    waits = list(si.on_wait)
                    for w in waits[:-1]:
                        n = mybir.InstNoOp(
                            name=f"I-wsplit{_WSPLIT_ID[0]}", ins=[], outs=[])
                        _WSPLIT_ID[0] += 1
                        n.engine = inst.engine
                        n.sync_info = mybir.SyncInfo(on_wait=[w], on_update=[])
                        out.append(n)
                    inst.sync_info = mybir.SyncInfo(
                        on_wait=[waits[-1]],
                        on_update=list(si.on_update or []))
                out.append(inst)
            if changed:
                bb.instructions = out


P = 128


def build_moe(DC=8, FC=12, E=8, NLOC=2048, TT=256, split_waits=True, repeat=1):
    """Build the per-core Bass module.

    DC: contraction chunks (D = DC*128); FC: half-ffn chunks (HALF = FC*128);
    E: routed experts (UNITS = E+1, last is shared); NLOC: tokens per core;
    TT: token tile for the expert sweep.
    """
    UNITS = E + 1
    D = DC * P
    ntt = NLOC // TT
    nt128 = NLOC // P

    nc = bass.Bass(target_bir_lowering=False)
    xt = nc.declare_dram_parameter("xt", [P, DC, NLOC], F32, isOutput=False)
    xtb = nc.declare_dram_parameter("xtb", [P, DC, NLOC], BF, isOutput=False)
    gt = nc.declare_dram_parameter("gt", [P, DC, E], F32, isOutput=False)
    wg = nc.declare_dram_parameter("wg", [UNITS, FC, P, DC, P], BF, isOutput=False)
    wu = nc.declare_dram_parameter("wu", [UNITS, FC, P, DC, P], BF, isOutput=False)
    wd = nc.declare_dram_parameter("wd", [UNITS, FC, P, D], BF, isOutput=False)
    ident = nc.declare_dram_parameter("ident", [P, P], F32, isOutput=False)
    outp = nc.declare_dram_parameter("out", [P, DC, NLOC], F32, isOutput=True)
    combT_dram = nc.dram_tensor("combT_dram", [UNITS, nt128, P], BF)

    with tile.TileContext(nc) as tc:
      for _rep in range(repeat):
        with ExitStack() as ctx:
            # long-lived tiles
            const_pool = ctx.enter_context(tc.tile_pool(name="const", bufs=1))
            xtb_sb = const_pool.tile([P, DC, NLOC], BF)
            nc.sync.dma_start(xtb_sb[:], xtb[:, :, :])
            acc_sb = const_pool.tile([P, DC, NLOC], F32)

            # ---------------- Phase A: routing ----------------
            with ExitStack() as actx:
                apool = actx.enter_context(tc.tile_pool(name="routeA", bufs=1))
                rpool = actx.enter_context(tc.tile_pool(name="routeR", bufs=2))
                apsum = actx.enter_context(
                    tc.tile_pool(name="routeP", bufs=2, space="PSUM"))

                comb_sb = apool.tile([P, nt128, UNITS], F32)
                combT_sb = apool.tile([UNITS, nt128, P], BF)
                xt_sb = apool.tile([P, DC, NLOC], F32)
                nc.sync.dma_start(xt_sb[:], xt[:, :, :])
                gt_sb = apool.tile([P, DC, E], F32)
                nc.sync.dma_start(gt_sb[:], gt[:, :, :])
                id_sb = apool.tile([P, P], F32)
                nc.sync.dma_start(id_sb[:], ident[:, :])

                # shared-expert combine weight is 1
                nc.vector.memset(comb_sb[:, :, E], 1.0)

                for t in range(nt128):
                    ps_l = apsum.tile([P, E], F32, tag="psl")
                    for c in range(DC):
                        nc.tensor.matmul(
                            ps_l[:],
                            xt_sb[:, c, bass.ts(t, P)],
                            gt_sb[:, c, :],
                            start=(c == 0), stop=(c == DC - 1),
                        )
                    lt = rpool.tile([P, E], F32, tag="lt")
                    nc.scalar.copy(lt[:], ps_l[:])
                    m1 = rpool.tile([P, 1], F32, tag="m1")
                    nc.vector.reduce_max(m1[:], lt[:], axis=AX.X)
                    eq = rpool.tile([P, E], F32, tag="eq")
                    nc.vector.tensor_scalar(
                        eq[:], lt[:], m1[:], None, op0=AluOpType.is_equal)
                    l2 = rpool.tile([P, E], F32, tag="l2")
                    nc.vector.scalar_tensor_tensor(
                        l2[:], eq[:], -1e30, lt[:],
                        op0=AluOpType.mult, op1=AluOpType.add)
                    m2 = rpool.tile([P, 1], F32, tag="m2")
                    nc.vector.reduce_max(m2[:], l2[:], axis=AX.X)
                    nm1 = rpool.tile([P, 1], F32, tag="nm1")
                    nc.vector.tensor_scalar_mul(nm1[:], m1[:], -1.0)
                    ex = rpool.tile([P, E], F32, tag="ex")
                    nc.scalar.activation(ex[:], lt[:], AF.Exp, bias=nm1[:], scale=1.0)
                    mk = rpool.tile([P, E], F32, tag="mk")
                    nc.vector.tensor_scalar(
                        mk[:], lt[:], m2[:], None, op0=AluOpType.is_ge)
                    we = rpool.tile([P, E], F32, tag="we")
                    nc.vector.tensor_tensor(we[:], ex[:], mk[:], op=AluOpType.mult)
                    s = rpool.tile([P, 1], F32, tag="s")
                    nc.vector.reduce_sum(s[:], we[:], axis=AX.X)
                    rs = rpool.tile([P, 1], F32, tag="rs")
                    nc.vector.reciprocal(rs[:], s[:])
                    nc.vector.tensor_scalar(
                        comb_sb[:, t, 0:E], we[:], rs[:], None, op0=AluOpType.mult)

                    # comb tile [128, UNITS] -> combT [UNITS, 128]
                    ps_t = apsum.tile([P, P], F32, tag="pst")
                    nc.tensor.transpose(
                        ps_t[0:UNITS, :], comb_sb[:, t, :], id_sb[:])
                    nc.scalar.copy(combT_sb[:, t, :], ps_t[0:UNITS, :])
                nc.sync.dma_start(combT_dram[:, :, :], combT_sb[:])

            # ---------------- Phase B: experts ----------------
            with ExitStack() as bctx:
                wpool = bctx.enter_context(tc.tile_pool(name="wpool", bufs=1))
                cpool = bctx.enter_context(tc.tile_pool(name="cpool", bufs=1))
                hpool = bctx.enter_context(tc.tile_pool(name="hpool", bufs=2))
                spool = bctx.enter_context(tc.tile_pool(name="spool", bufs=2))
                gpsum = bctx.enter_context(
                    tc.tile_pool(name="gpsum", bufs=2, space="PSUM"))
                upsum = bctx.enter_context(
                    tc.tile_pool(name="upsum", bufs=2, space="PSUM"))
                ypsum = bctx.enter_context(
                    tc.tile_pool(name="ypsum", bufs=1, space="PSUM"))

                for u in range(UNITS):
                    wg_sb = wpool.tile([P, FC, DC, P], BF, tag="wg")
                    wu_sb = wpool.tile([P, FC, DC, P], BF, tag="wu")
                    wd_sb = wpool.tile([P, FC, D], BF, tag="wd")
                    for fc in range(FC):
                        nc.sync.dma_start(wg_sb[:, fc], wg[u, fc])
                        nc.sync.dma_start(wu_sb[:, fc], wu[u, fc])
                        nc.sync.dma_start(wd_sb[:, fc], wd[u, fc])

                    cb_u = cpool.tile([P, NLOC], BF, tag="cb")
                    nc.sync.dma_start(
                        cb_u[:],
                        combT_dram[u:u + 1].partition_broadcast(P).opt())

                    for tt in range(ntt):
                        hs_sb = hpool.tile([P, FC, TT], BF, tag="hs")
                        ps_y = ypsum.tile([P, DC * TT], F32, tag="py")
                        for fc in range(FC):
                            ps_g = gpsum.tile([P, TT], F32, tag="pg")
                            ps_u = upsum.tile([P, TT], F32, tag="pu")
                            for c in range(DC):
                                nc.tensor.matmul(
                                    ps_g[:],
                                    wg_sb[:, fc, c, :],
                                    xtb_sb[:, c, bass.ts(tt, TT)],
                                    start=(c == 0), stop=(c == DC - 1),
                                )
                            for c in range(DC):
                                nc.tensor.matmul(
                                    ps_u[:],
                                    wu_sb[:, fc, c, :],
                                    xtb_sb[:, c, bass.ts(tt, TT)],
                                    start=(c == 0), stop=(c == DC - 1),
                                )
                            sg = spool.tile([P, TT], F32, tag="sg")
                            nc.scalar.activation(sg[:], ps_g[:], AF.Silu)
                            h = spool.tile([P, TT], F32, tag="h")
                            nc.vector.tensor_tensor(
                                h[:], sg[:], ps_u[:], op=AluOpType.mult)
                            nc.vector.tensor_tensor(
                                hs_sb[:, fc, :], h[:], cb_u[:, bass.ts(tt, TT)],
                                op=AluOpType.mult)
                        for dcc in range(DC):
                            for fc in range(FC):
                                nc.tensor.matmul(
                                    ps_y[:, bass.ts(dcc, TT)],
                                    wd_sb[:, fc, bass.ts(dcc, P)],
                                    hs_sb[:, fc, :],
                                    start=(fc == 0), stop=(fc == FC - 1),
                                )
                        ps_y_v = ps_y[:].rearrange("p (c t) -> p c t", c=DC)
                        if u == 0:
                            nc.vector.tensor_copy(
                                acc_sb[:, :, bass.ts(tt, TT)], ps_y_v)
                        else:
                            nc.vector.tensor_tensor(
                                acc_sb[:, :, bass.ts(tt, TT)],
                                acc_sb[:, :, bass.ts(tt, TT)],
                                ps_y_v, op=AluOpType.add)

            nc.sync.dma_start(outp[:, :, :], acc_sb[:])
    if split_waits:
        _split_multi_waits(nc)
    return nc


def build_moe_v2(DC=8, FC=12, E=8, NLOC=2048, split_waits=True, repeat=1):
    """Dense v2: down-projection uses hs as the stationary operand with the
    full model dim as the moving axis (N=512 matmuls, half the instruction
    count of v1's N=256 form), output lands token-major, and the combine
    weight is applied in one fused multiply-add per unit on the DVE."""
    UNITS = E + 1
    D = DC * P
    nt128 = NLOC // P

    nc = bass.Bass(target_bir_lowering=False)
    xt = nc.declare_dram_parameter("xt", [P, DC, NLOC], F32, isOutput=False)
    xtb = nc.declare_dram_parameter("xtb", [P, DC, NLOC], BF, isOutput=False)
    gt = nc.declare_dram_parameter("gt", [P, DC, E], F32, isOutput=False)
    wg = nc.declare_dram_parameter("wg", [UNITS, FC, P, DC, P], BF, isOutput=False)
    wu = nc.declare_dram_parameter("wu", [UNITS, FC, P, DC, P], BF, isOutput=False)
    wd = nc.declare_dram_parameter("wd", [UNITS, FC, P, D], BF, isOutput=False)
    ident = nc.declare_dram_parameter("ident", [P, P], F32, isOutput=False)
    outp = nc.declare_dram_parameter("out", [NLOC, D], F32, isOutput=True)

    with tile.TileContext(nc) as tc:
      for _rep in range(repeat):
        with ExitStack() as ctx:
            const_pool = ctx.enter_context(tc.tile_pool(name="const", bufs=1))
            xtb_sb = const_pool.tile([P, DC, NLOC], BF)
            nc.sync.dma_start(xtb_sb[:], xtb[:, :, :])
            acc_sb = const_pool.tile([P, nt128, D], F32)
            comb_sb = const_pool.tile([P, nt128, UNITS], F32)
            nc.vector.memset(comb_sb[:, :, E], 1.0)

            # ---------------- Phase A: routing ----------------
            with ExitStack() as actx:
                apool = actx.enter_context(tc.tile_pool(name="routeA", bufs=1))
                rpool = actx.enter_context(tc.tile_pool(name="routeR", bufs=2))
                apsum = actx.enter_context(
                    tc.tile_pool(name="routeP", bufs=2, space="PSUM"))

                xt_sb = apool.tile([P, DC, NLOC], F32)
                nc.sync.dma_start(xt_sb[:], xt[:, :, :])
                gt_sb = apool.tile([P, DC, E], F32)
                nc.sync.dma_start(gt_sb[:], gt[:, :, :])
                id_sb = apool.tile([P, P], F32)
                nc.sync.dma_start(id_sb[:], ident[:, :])

                for t in range(nt128):
                    ps_l = apsum.tile([P, E], F32, tag="psl")
                    for c in range(DC):
                        nc.tensor.matmul(
                            ps_l[:],
                            xt_sb[:, c, bass.ts(t, P)],
                            gt_sb[:, c, :],
                            start=(c == 0), stop=(c == DC - 1),
                        )
                    lt = rpool.tile([P, E], F32, tag="lt")
                    nc.scalar.copy(lt[:], ps_l[:])
                    m1 = rpool.tile([P, 1], F32, tag="m1")
                    nc.vector.reduce_max(m1[:], lt[:], axis=AX.X)
                    eq = rpool.tile([P, E], F32, tag="eq")
                    nc.vector.tensor_scalar(
                        eq[:], lt[:], m1[:], None, op0=AluOpType.is_equal)
                    l2 = rpool.tile([P, E], F32, tag="l2")
                    nc.vector.scalar_tensor_tensor(
                        l2[:], eq[:], -1e30, lt[:],
                        op0=AluOpType.mult, op1=AluOpType.add)
                    m2 = rpool.tile([P, 1], F32, tag="m2")
                    nc.vector.reduce_max(m2[:], l2[:], axis=AX.X)
                    nm1 = rpool.tile([P, 1], F32, tag="nm1")
                    nc.vector.tensor_scalar_mul(nm1[:], m1[:], -1.0)
                    ex = rpool.tile([P, E], F32, tag="ex")
                    nc.scalar.activation(ex[:], lt[:], AF.Exp, bias=nm1[:], scale=1.0)
                    mk = rpool.tile([P, E], F32, tag="mk")
                    nc.vector.tensor_scalar(
                        mk[:], lt[:], m2[:], None, op0=AluOpType.is_ge)
                    we = rpool.tile([P, E], F32, tag="we")
                    nc.vector.tensor_tensor(we[:], ex[:], mk[:], op=AluOpType.mult)
                    s = rpool.tile([P, 1], F32, tag="s")
                    nc.vector.reduce_sum(s[:], we[:], axis=AX.X)
                    rs = rpool.tile([P, 1], F32, tag="rs")
                    nc.vector.reciprocal(rs[:], s[:])
                    nc.vector.tensor_scalar(
                        comb_sb[:, t, 0:E], we[:], rs[:], None, op0=AluOpType.mult)

            # ---------------- Phase B: experts ----------------
            with ExitStack() as bctx:
                wpool = bctx.enter_context(tc.tile_pool(name="wpool", bufs=1))
                hpool = bctx.enter_context(tc.tile_pool(name="hpool", bufs=3))
                spool = bctx.enter_context(tc.tile_pool(name="spool", bufs=2))
                gpsum = bctx.enter_context(
                    tc.tile_pool(name="gpsum", bufs=2, space="PSUM"))
                upsum = bctx.enter_context(
                    tc.tile_pool(name="upsum", bufs=2, space="PSUM"))
                ypsum = bctx.enter_context(
                    tc.tile_pool(name="ypsum", bufs=2, space="PSUM"))

                dw = min(512, D)
                for u in range(UNITS):
                    wg_sb = wpool.tile([P, FC, DC, P], BF, tag="wg")
                    wu_sb = wpool.tile([P, FC, DC, P], BF, tag="wu")
                    wd_sb = wpool.tile([P, FC, D], BF, tag="wd")
                    for fc in range(FC):
                        nc.sync.dma_start(wg_sb[:, fc], wg[u, fc])
                        nc.scalar.dma_start(wu_sb[:, fc], wu[u, fc])
                        nc.gpsimd.dma_start(wd_sb[:, fc], wd[u, fc])

                    # g/u at the widest moving dim the psum budget allows;
                    # hs stays 256-wide for the down stage
                    TW = 512 if NLOC % 512 == 0 else 256
                    NH = TW // 256
                    for tt in range(NLOC // TW):
                        ps_g = gpsum.tile([P, TW], F32, tag="pg")
                        ps_u = upsum.tile([P, TW], F32, tag="pu")
                        hs_tiles = [
                            hpool.tile([P, FC, 256], BF, tag="hs",
                                       name=f"hs_{u}_{tt}_{h}")
                            for h in range(NH)]
                        for fc in range(FC):
                            for c in range(DC):
                                nc.tensor.matmul(
                                    ps_g[:], wg_sb[:, fc, c, :],
                                    xtb_sb[:, c, bass.ts(tt, TW)],
                                    start=(c == 0), stop=(c == DC - 1))
                            for c in range(DC):
                                nc.tensor.matmul(
                                    ps_u[:], wu_sb[:, fc, c, :],
                                    xtb_sb[:, c, bass.ts(tt, TW)],
                                    start=(c == 0), stop=(c == DC - 1))
                            sg_t = spool.tile([P, TW], F32, tag="sg")
                            nc.scalar.activation(sg_t[:], ps_g[:], AF.Silu)
                            for h in range(NH):
                                nc.vector.tensor_tensor(
                                    hs_tiles[h][:, fc, :],
                                    sg_t[:, h * 256:(h + 1) * 256],
                                    ps_u[:, h * 256:(h + 1) * 256],
                                    op=AluOpType.mult)
                        for h in range(NH):
                            for sub in range(2):
                                t128 = (tt * NH + h) * 2 + sub
                                yp = ypsum.tile([P, D], F32, tag="yp")
                                for half in range(D // dw):
                                    for fc in range(FC):
                                        nc.tensor.matmul(
                                            yp[:, half * dw:(half + 1) * dw],
                                            hs_tiles[h][:, fc,
                                                        sub * P:(sub + 1) * P],
                                            wd_sb[:, fc,
                                                  half * dw:(half + 1) * dw],
                                            start=(fc == 0),
                                            stop=(fc == FC - 1))
                                if u == 0:
                                    nc.vector.tensor_scalar(
                                        acc_sb[:, t128, :], yp[:],
                                        comb_sb[:, t128, u:u + 1], None,
                                        op0=AluOpType.mult)
                                else:
                                    nc.vector.scalar_tensor_tensor(
                                        acc_sb[:, t128, :], yp[:],
                                        comb_sb[:, t128, u:u + 1],
                                        acc_sb[:, t128, :],
                                        op0=AluOpType.mult, op1=AluOpType.add)

            nc.sync.dma_start(
                outp[:, :].rearrange("(t p) d -> p t d", p=P), acc_sb[:])
    if split_waits:
        _split_multi_waits(nc)
    return nc


def build_moe_sparse(DC=8, FC=12, E=8, NLOC=2048, CAP=640, split_waits=True,
                     repeat=1):
    """Sparse (top-2 gathered) variant.

    Routing runs as in the dense kernel; per expert, selected token indices
    are compacted on device (sparse_gather), token activations are gathered
    transposed straight into the d-chunked matmul layout (dma_gather), the
    expert SwiGLU runs only on CAP capacity slots, outputs are scaled by the
    gathered combine weight and scatter-added into a token-major DRAM
    accumulator primed by the shared expert. Pad slots point at a zeroed
    dummy token row so every op stays static-shape.
    """
    from concourse import library_config

    UNITS = E + 1
    D = DC * P
    nt128 = NLOC // P
    F16 = NLOC // 16
    CW = CAP // 16
    NST = CAP // P
    DUMMY = NLOC  # index of the zeroed dummy row
    ST_LIST = []
    s0 = 0
    while s0 < CAP:
        w = min(256, CAP - s0)
        ST_LIST.append((s0, w))
        s0 += w

    nc = bass.Bass(target_bir_lowering=False)
    xt = nc.declare_dram_parameter("xt", [P, DC, NLOC], F32, isOutput=False)
    xtb = nc.declare_dram_parameter("xtb", [P, DC, NLOC], BF, isOutput=False)
    xb = nc.declare_dram_parameter("xb", [NLOC + 16, D], BF, isOutput=False)
    gt = nc.declare_dram_parameter("gt", [P, DC, E], F32, isOutput=False)
    wg = nc.declare_dram_parameter("wg", [UNITS, FC, P, DC, P], BF, isOutput=False)
    wu = nc.declare_dram_parameter("wu", [UNITS, FC, P, DC, P], BF, isOutput=False)
    wd = nc.declare_dram_parameter("wd", [UNITS, FC, P, D], BF, isOutput=False)
    ident = nc.declare_dram_parameter("ident", [P, P], F32, isOutput=False)
    iota16 = nc.declare_dram_parameter("iota16", [16, F16], F32, isOutput=False)
    outp = nc.declare_dram_parameter("out", [NLOC, D], F32, isOutput=True)
    combT_dram = nc.dram_tensor("combT_dram", [E, nt128, P], BF)
    combR_dram = nc.dram_tensor("combR_dram", [E, NLOC + 16, 64], F32)
    acc_dram = nc.dram_tensor("acc_dram", [NLOC + 16, D], F32)

    with tile.TileContext(nc) as tc:
      for _rep in range(repeat):
        with ExitStack() as ctx:
            const_pool = ctx.enter_context(tc.tile_pool(name="const", bufs=1))
            xtb_sb = const_pool.tile([P, DC, NLOC], BF)
            nc.sync.dma_start(xtb_sb[:], xtb[:, :, :])
            idx_sb = const_pool.tile([P, E, CW], mybir.dt.int16)

            # ---------------- Phase A: routing + index build ----------------
            with ExitStack() as actx:
                apool = actx.enter_context(tc.tile_pool(name="routeA", bufs=1))
                rpool = actx.enter_context(tc.tile_pool(name="routeR", bufs=2))
                apsum = actx.enter_context(
                    tc.tile_pool(name="routeP", bufs=2, space="PSUM"))

                comb_sb = apool.tile([P, nt128, E], F32)
                combT_sb = apool.tile([E, nt128, P], BF)
                xt_sb = apool.tile([P, DC, NLOC], F32)
                nc.sync.dma_start(xt_sb[:], xt[:, :, :])
                gt_sb = apool.tile([P, DC, E], F32)
                nc.sync.dma_start(gt_sb[:], gt[:, :, :])
                id_sb = apool.tile([P, P], F32)
                nc.sync.dma_start(id_sb[:], ident[:, :])
                iota_sb = apool.tile([16, F16], F32)
                nc.sync.dma_start(iota_sb[:], iota16[:, :])
                neg1_sb = apool.tile([16, F16], F32)
                nc.vector.memset(neg1_sb[:], -1.0)
                dummy_sb = apool.tile([16, F16], F32)
                nc.vector.memset(dummy_sb[:], float(DUMMY))
                zz_sb = apool.tile([16, 64], F32)
                nc.vector.memset(zz_sb[:], 0.0)

                for t in range(nt128):
                    ps_l = apsum.tile([P, E], F32, tag="psl")
                    for c in range(DC):
                        nc.tensor.matmul(
                            ps_l[:],
                            xt_sb[:, c, bass.ts(t, P)],
                            gt_sb[:, c, :],
                            start=(c == 0), stop=(c == DC - 1),
                        )
                    lt = rpool.tile([P, E], F32, tag="lt")
                    nc.scalar.copy(lt[:], ps_l[:])
                    m1 = rpool.tile([P, 1], F32, tag="m1")
                    nc.vector.reduce_max(m1[:], lt[:], axis=AX.X)
                    eq = rpool.tile([P, E], F32, tag="eq")
                    nc.vector.tensor_scalar(
                        eq[:], lt[:], m1[:], None, op0=AluOpType.is_equal)
                    l2 = rpool.tile([P, E], F32, tag="l2")
                    nc.vector.scalar_tensor_tensor(
                        l2[:], eq[:], -1e30, lt[:],
                        op0=AluOpType.mult, op1=AluOpType.add)
                    m2 = rpool.tile([P, 1], F32, tag="m2")
                    nc.vector.reduce_max(m2[:], l2[:], axis=AX.X)
                    nm1 = rpool.tile([P, 1], F32, tag="nm1")
                    nc.vector.tensor_scalar_mul(nm1[:], m1[:], -1.0)
                    ex = rpool.tile([P, E], F32, tag="ex")
                    nc.scalar.activation(ex[:], lt[:], AF.Exp, bias=nm1[:], scale=1.0)
                    mk = rpool.tile([P, E], F32, tag="mk")
                    nc.vector.tensor_scalar(
                        mk[:], lt[:], m2[:], None, op0=AluOpType.is_ge)
                    we = rpool.tile([P, E], F32, tag="we")
                    nc.vector.tensor_tensor(we[:], ex[:], mk[:], op=AluOpType.mult)
                    s = rpool.tile([P, 1], F32, tag="s")
                    nc.vector.reduce_sum(s[:], we[:], axis=AX.X)
                    rs = rpool.tile([P, 1], F32, tag="rs")
                    nc.vector.reciprocal(rs[:], s[:])
                    nc.vector.tensor_scalar(
                        comb_sb[:, t, :], we[:], rs[:], None, op0=AluOpType.mult)

                    ps_t = apsum.tile([P, P], F32, tag="pst")
                    nc.tensor.transpose(
                        ps_t[0:E, :], comb_sb[:, t, :], id_sb[:])
                    nc.scalar.copy(combT_sb[:, t, :], ps_t[0:E, :])
                nc.sync.dma_start(combT_dram[:, :, :], combT_sb[:])

                # combine-weight gather table (64-wide replicated, f32),
                # plus zeroed dummy rows
                for e in range(E):
                    crv = rpool.tile([P, nt128, 64], F32, tag="crv")
                    nc.vector.tensor_copy(
                        crv[:], comb_sb[:, :, e:e + 1].to_broadcast(
                            (P, nt128, 64)))
                    nc.sync.dma_start(
                        combR_dram[e, 0:NLOC].rearrange(
                            "(t p) r -> p t r", p=P),
                        crv[:])
                    nc.sync.dma_start(
                        combR_dram[e, NLOC:NLOC + 16], zz_sb[:])
                # zero acc dummy rows (scatter-add RMW reads them)
                zd_sb = apool.tile([16, D], F32)
                nc.vector.memset(zd_sb[:], 0.0)
                nc.sync.dma_start(acc_dram[NLOC:NLOC + 16, :], zd_sb[:])

                # per-expert index compaction
                lib_sg = nc.gpsimd.engine_nop()
                sg_insts = []
                for e in range(E):
                    mv = rpool.tile([16, F16], BF, tag="mv")
                    nc.sync.dma_start(
                        mv[:],
                        combT_dram[e].rearrange("a b -> (a b)").rearrange(
                            "(f q) -> q f", q=16))
                    msk = rpool.tile([16, F16], mybir.dt.uint8, tag="msk")
                    nc.vector.tensor_scalar(
                        msk[:], mv[:], 0.0, None, op0=AluOpType.is_gt)
                    tokneg = rpool.tile([16, F16], F32, tag="tokneg")
                    nc.vector.select(tokneg[:], msk[:], iota_sb[:], neg1_sb[:])
                    idxf = rpool.tile([16, F16], F32, tag="idxf")
                    nf = rpool.tile([1, 1], mybir.dt.uint32, tag="nf")
                    sg = nc.gpsimd.sparse_gather(
                        idxf[:], tokneg[:], num_found=nf[:])
                    tile.add_dep_helper(sg.ins, lib_sg.ins,
                                        reason="sparse_gather lib")
                    sg_insts.append(sg)
                    ge0 = rpool.tile([16, F16], mybir.dt.uint8, tag="ge0")
                    nc.vector.tensor_scalar(
                        ge0[:], idxf[:], 0.0, None, op0=AluOpType.is_ge)
                    idcl = rpool.tile([16, F16], F32, tag="idcl")
                    nc.vector.select(idcl[:], ge0[:], idxf[:], dummy_sb[:])
                    idci = rpool.tile([16, F16], mybir.dt.int16, tag="idci")
                    nc.vector.tensor_copy(idci[:], idcl[:])
                    for g in range(8):
                        nc.sync.dma_start(
                            idx_sb[16 * g:16 * (g + 1), e, :], idci[:, 0:CW])

                lib_mlp = nc.gpsimd.engine_nop()
                for sg in sg_insts:
                    tile.add_dep_helper(lib_mlp.ins, sg.ins,
                                        reason="mlp lib after sparse_gather")

            # ---------------- Phase B: experts ----------------
            with ExitStack() as bctx:
                wpool = bctx.enter_context(tc.tile_pool(name="wpool", bufs=1))
                xgpool = bctx.enter_context(tc.tile_pool(name="xgpool", bufs=2))
                cgpool = bctx.enter_context(tc.tile_pool(name="cgpool", bufs=2))
                hpool = bctx.enter_context(tc.tile_pool(name="hpool", bufs=2))
                spool = bctx.enter_context(tc.tile_pool(name="spool", bufs=2))
                ypool = bctx.enter_context(tc.tile_pool(name="ypool", bufs=1))
                gpsum = bctx.enter_context(
                    tc.tile_pool(name="gpsum", bufs=2, space="PSUM"))
                upsum = bctx.enter_context(
                    tc.tile_pool(name="upsum", bufs=2, space="PSUM"))
                ypsum = bctx.enter_context(
                    tc.tile_pool(name="ypsum", bufs=2, space="PSUM"))

                def load_unit_weights(u):
                    wg_sb = wpool.tile([P, FC, DC, P], BF, tag="wg")
                    wu_sb = wpool.tile([P, FC, DC, P], BF, tag="wu")
                    wd_sb = wpool.tile([P, FC, D], BF, tag="wd")
                    for fc in range(FC):
                        nc.sync.dma_start(wg_sb[:, fc], wg[u, fc])
                        nc.sync.dma_start(wu_sb[:, fc], wu[u, fc])
                        nc.sync.dma_start(wd_sb[:, fc], wd[u, fc])
                    return wg_sb, wu_sb, wd_sb

                def gu_sweep(wg_sb, wu_sb, rhs_fn, width):
                    # one token/slot tile: returns hs [P, FC, width] bf16
                    hs_sb = hpool.tile([P, FC, 256], BF, tag="hs")
                    for fc in range(FC):
                        ps_g = gpsum.tile([P, 256], F32, tag="pg")
                        ps_u = upsum.tile([P, 256], F32, tag="pu")
                        for c in range(DC):
                            nc.tensor.matmul(
                                ps_g[:, 0:width], wg_sb[:, fc, c, :], rhs_fn(c),
                                start=(c == 0), stop=(c == DC - 1))
                        for c in range(DC):
                            nc.tensor.matmul(
                                ps_u[:, 0:width], wu_sb[:, fc, c, :], rhs_fn(c),
                                start=(c == 0), stop=(c == DC - 1))
                        sg_t = spool.tile([P, 256], F32, tag="sg")
                        nc.scalar.activation(
                            sg_t[:, 0:width], ps_g[:, 0:width], AF.Silu)
                        nc.vector.tensor_tensor(
                            hs_sb[:, fc, 0:width], sg_t[:, 0:width],
                            ps_u[:, 0:width], op=AluOpType.mult)
                    return hs_sb

                def down_sub(hs_sb, wd_sb, sub):
                    # one 128-slot subtile -> psum [P, D]
                    yp = ypsum.tile([P, D], F32, tag="yp")
                    dw = min(512, D)
                    for half in range(D // dw):
                        for fc in range(FC):
                            nc.tensor.matmul(
                                yp[:, half * dw:(half + 1) * dw],
                                hs_sb[:, fc, sub * P:(sub + 1) * P],
                                wd_sb[:, fc, half * dw:(half + 1) * dw],
                                start=(fc == 0), stop=(fc == FC - 1))
                    return yp

                # shared expert: dense over all tokens, direct row writes
                wg_sb, wu_sb, wd_sb = load_unit_weights(E)
                for tt in range(NLOC // 256):
                    hs_sb = gu_sweep(
                        wg_sb, wu_sb,
                        lambda c: xtb_sb[:, c, bass.ts(tt, 256)], 256)
                    for sub in range(2):
                        yp = down_sub(hs_sb, wd_sb, sub)
                        ysh = spool.tile([P, D], F32, tag="ysh")
                        nc.scalar.copy(ysh[:], yp[:])
                        r0 = (tt * 2 + sub) * P
                        nc.sync.dma_start(acc_dram[r0:r0 + P, :], ysh[:])

                # routed experts: gathered slots, comb-scaled scatter-add
                for e in range(E):
                    wg_sb, wu_sb, wd_sb = load_unit_weights(e)
                    xg_sb = xgpool.tile([P, DC, CAP], BF, tag="xg")
                    g1 = nc.gpsimd.dma_gather(
                        xg_sb[:], xb[:, :], idx_sb[:, e, :],
                        num_idxs=CAP, num_idxs_reg=CAP, elem_size=D,
                        transpose=True)
                    tile.add_dep_helper(g1.ins, lib_mlp.ins, reason="mlp lib")
                    cg_sb = cgpool.tile([P, NST, 64], F32, tag="cg")
                    g2 = nc.gpsimd.dma_gather(
                        cg_sb[:], combR_dram[e], idx_sb[:, e, :],
                        num_idxs=CAP, num_idxs_reg=CAP, elem_size=64,
                        transpose=False)
                    tile.add_dep_helper(g2.ins, lib_mlp.ins, reason="mlp lib")

                    ysc = ypool.tile([P, NST, D], F32, tag="ysc")
                    for (s0, sw) in ST_LIST:
                        hs_sb = gu_sweep(
                            wg_sb, wu_sb,
                            lambda c: xg_sb[:, c, s0:s0 + sw], sw)
                        for sub in range(sw // P):
                            gsub = s0 // P + sub
                            yp = down_sub(hs_sb, wd_sb, sub)
                            nc.vector.tensor_scalar(
                                ysc[:, gsub, :], yp[:], cg_sb[:, gsub, 0:1],
                                None, op0=AluOpType.mult)
                    sc = nc.gpsimd.dma_scatter_add(
                        acc_dram[:, :], ysc[:], idx_sb[:, e, :],
                        num_idxs=CAP, num_idxs_reg=CAP, elem_size=D)
                    tile.add_dep_helper(sc.ins, lib_mlp.ins, reason="mlp lib")

                nc.sync.dma_start(outp[:, :], acc_dram[0:NLOC, :])
    if split_waits:
        _split_multi_waits(nc)
    return nc


# ---------------------------------------------------------------------------
# Host side


def _prep_weight_gu(w, DC, FC):
    # w [HALF, D] -> [FC, 128, DC, 128]: out[fc, p, c, f] = w[fc*128+f, c*128+p]
    D = DC * P
    HALF = FC * P
    wt = w.T.reshape(DC, P, FC, P).transpose(2, 1, 0, 3)
    return np.ascontiguousarray(wt.astype(bf16))


def _prep_weight_d(w, DC, FC):
    # w [D, HALF] -> [FC, 128, D]: out[fc, p, d] = w[d, fc*128+p]
    wt = w.T.reshape(FC, P, DC * P)
    return np.ascontiguousarray(wt.astype(bf16))


_BUILT = {}

USE_SPARSE = False
USE_V2 = True


def _get_built(key, **kw):
    if key not in _BUILT:
        if USE_SPARSE:
            _BUILT[key] = build_moe_sparse(**kw)
        elif USE_V2:
            _BUILT[key] = build_moe_v2(**kw)
        else:
            _BUILT[key] = build_moe(**kw)
    return _BUILT[key]


def prepare(x, gate_w, w_up, w_down, sg_gate, sg_up, sg_down):
    """Build (nc, in_maps, meta) for the 8-core SPMD launch."""
    B, T, D = x.shape
    E = gate_w.shape[0]
    FFN = w_up.shape[1]
    HALF = FFN // 2
    DC = D // P
    FC = HALF // P
    N = B * T
    NCORES = 8
    NLOC = N // NCORES

    nc = _get_built((DC, FC, E, NLOC), DC=DC, FC=FC, E=E, NLOC=NLOC)

    UNITS = E + 1
    wg_all = np.empty((UNITS, FC, P, DC, P), dtype=bf16)
    wu_all = np.empty((UNITS, FC, P, DC, P), dtype=bf16)
    wd_all = np.empty((UNITS, FC, P, D), dtype=bf16)
    for u in range(E):
        wg_all[u] = _prep_weight_gu(w_up[u, :HALF], DC, FC)
        wu_all[u] = _prep_weight_gu(w_up[u, HALF:], DC, FC)
        wd_all[u] = _prep_weight_d(w_down[u], DC, FC)
    wg_all[E] = _prep_weight_gu(sg_gate, DC, FC)
    wu_all[E] = _prep_weight_gu(sg_up, DC, FC)
    wd_all[E] = _prep_weight_d(sg_down, DC, FC)

    gt = np.ascontiguousarray(
        gate_w.T.reshape(DC, P, E).transpose(1, 0, 2).astype(np.float32))
    ident = np.eye(P, dtype=np.float32)
    F16 = NLOC // 16
    iota16 = np.ascontiguousarray(
        (np.arange(F16)[None, :] * 16 + np.arange(16)[:, None])
        .astype(np.float32))

    xf = np.ascontiguousarray(x.reshape(N, D))
    in_maps = []
    for ci in range(NCORES):
        xc = xf[ci * NLOC:(ci + 1) * NLOC]
        xt = np.ascontiguousarray(
            xc.T.reshape(DC, P, NLOC).transpose(1, 0, 2).astype(np.float32))
        xtb = xt.astype(bf16)
        m = {
            "xt": xt, "xtb": xtb, "gt": gt,
            "wg": wg_all, "wu": wu_all, "wd": wd_all,
            "ident": ident,
        }
        if USE_SPARSE:
            xbp = np.zeros((NLOC + 16, D), dtype=bf16)
            xbp[:NLOC] = xc.astype(bf16)
            m["xb"] = xbp
            m["iota16"] = iota16
        in_maps.append(m)

    return nc, in_maps, (B, T, D, NLOC, NCORES)


def postprocess(results, meta):
    B, T, D, NLOC, NCORES = meta
    outs = []
    for ci in range(NCORES):
        o = results[ci]["out"]
        if USE_SPARSE or USE_V2:
            outs.append(o.reshape(NLOC, D))
        else:
            DC = D // P
            outs.append(
                o.reshape(P, DC, NLOC).transpose(1, 0, 2).reshape(D, NLOC).T)
    return np.concatenate(outs, axis=0).reshape(B, T, D).astype(np.float32)


def kernel(x, gate_w, w_up, w_down, sg_gate, sg_up, sg_down):
    from concourse.bass_utils import run_bass_kernel_spmd

    nc, in_maps, meta = prepare(
        x, gate_w, w_up, w_down, sg_gate, sg_up, sg_down)
    r = run_bass_kernel_spmd(nc, in_maps, core_ids=list(range(meta[4])))
    return postprocess(r.results, meta)



# revision 11
# speedup vs baseline: 1.8608x; 1.8608x over previous
"""DeepSeek-MoE FFN (8 routed experts, top-2, SwiGLU, shared expert) on 8
Trainium2 NeuronCores.

Strategy: token-parallel. Each core takes N/8 = 2048 tokens and computes the
full mixture for them (all 8 routed experts densely, weighted by the dense
combine matrix, plus the shared expert); no collectives. Routing (gate
logits, top-2, softmax) runs in fp32 on device; expert matmuls run in bf16
with fp32 PSUM accumulation.

Per-core layouts (host-prepped, d-chunked so every DMA line is contiguous):
  xt   [128, 8, 2048] f32   xt[p, c, t]  = x[t, c*128+p]      (gate matmul)
  xtb  [128, 8, 2048] bf16  same, bf16                        (expert matmuls)
  gt   [128, 8, 8]    f32   gt[p, c, e]  = gate_w[e, c*128+p]
  wg   [9, 12, 128, 8, 128] bf16  wg[u, fc, p, c, f] = Wg_u[fc*128+f, c*128+p]
  wu   same layout for the up projection
  wd   [9, 12, 128, 1024]   bf16  wd[u, fc, p, d]    = Wd_u[d, fc*128+p]
  (unit 8 is the shared expert; its combine weight is fixed at 1.0)
  out  [128, 8, 2048] f32   out[p, c, t] = y[t, c*128+p]
"""

import sys

if '/opt/trn_rl_repo' not in sys.path:
    sys.path.insert(0, '/opt/trn_rl_repo')

from contextlib import ExitStack

import numpy as np
import ml_dtypes

import concourse.bass as bass
import concourse.tile as tile
import concourse.mybir as mybir
from concourse.alu_op_type import AluOpType
from concourse.vector_clock import ScopedClock

bf16 = ml_dtypes.bfloat16
F32 = mybir.dt.float32
BF = mybir.dt.bfloat16
AF = mybir.ActivationFunctionType
AX = mybir.AxisListType

# ---------------------------------------------------------------------------
# TileContext tail-drain fix: the stock exit emits one Drain carrying a sem
# wait per live logical proc, but walrus only accepts a single sync wait per
# SP instruction. Split the waits across preceding sync nops.
_MAX_WAITS = 1


def _patched_drain_and_barrier(self, tick_clock, wait_clock):
    nc = self.nc
    probe = nc.sync.nop()
    wait_clock.add_sem_waits(probe.ins, ScopedClock({None: tick_clock.global_clock}))
    si = probe.ins.sync_info
    waits = list(si.on_wait) if si is not None else []
    if len(waits) > _MAX_WAITS:
        probe.ins.sync_info = mybir.SyncInfo(on_wait=waits[:_MAX_WAITS], on_update=[])
        for k in range(_MAX_WAITS, len(waits), _MAX_WAITS):
            n = nc.sync.nop()
            n.ins.sync_info = mybir.SyncInfo(
                on_wait=waits[k:k + _MAX_WAITS], on_update=[]
            )
    nc.sync.drain()
    nc.all_engine_barrier()
    assert self.sems is not None
    popped = nc._tile_sem_poison_stack.pop()
    assert popped is self._sem_poison
    nc.clear_and_free_semaphores(list(self.sems.allocated().values()))
    nc.all_engine_barrier()


tile.TileContext._drain_and_barrier = _patched_drain_and_barrier

# ---------------------------------------------------------------------------
# This walrus build accepts only ONE sync wait per instruction. Hoist extra
# waits onto standalone same-engine NoOps placed immediately before.
_WSPLIT_ID = [0]


def _split_multi_waits(nc):
    for f in nc.m.functions:
        for bb in f.blocks:
            out = []
            changed = False
            for inst in bb.instructions:
                si = getattr(inst, 'sync_info', None)
                if si is not None and si.on_wait and len(si.on_wait) > 1:
                    changed = True
                    waits = list(si.on_wait)
                    for w in waits[:-1]:
                        n = mybir.InstNoOp(
                            name=f"I-wsplit{_WSPLIT_ID[0]}", ins=[], outs=[])
                        _WSPLIT_ID[0] += 1
                        n.engine = inst.engine
                        n.sync_info = mybir.SyncInfo(on_wait=[w], on_update=[])
                        out.append(n)
                    inst.sync_info = mybir.SyncInfo(
                        on_wait=[waits[-1]],
                        on_update=list(si.on_update or []))
                out.append(inst)
            if changed:
                bb.instructions = out


P = 128


def build_moe(DC=8, FC=12, E=8, NLOC=2048, TT=256, split_waits=True, repeat=1):
    """Build the per-core Bass module.

    DC: contraction chunks (D = DC*128); FC: half-ffn chunks (HALF = FC*128);
    E: routed experts (UNITS = E+1, last is shared); NLOC: tokens per core;
    TT: token tile for the expert sweep.
    """
    UNITS = E + 1
    D = DC * P
    ntt = NLOC // TT
    nt128 = NLOC // P

    nc = bass.Bass(target_bir_lowering=False)
    xt = nc.declare_dram_parameter("xt", [P, DC, NLOC], F32, isOutput=False)
    xtb = nc.declare_dram_parameter("xtb", [P, DC, NLOC], BF, isOutput=False)
    gt = nc.declare_dram_parameter("gt", [P, DC, E], F32, isOutput=False)
    wg = nc.declare_dram_parameter("wg", [UNITS, FC, P, DC, P], BF, isOutput=False)
    wu = nc.declare_dram_parameter("wu", [UNITS, FC, P, DC, P], BF, isOutput=False)
    wd = nc.declare_dram_parameter("wd", [UNITS, FC, P, D], BF, isOutput=False)
    ident = nc.declare_dram_parameter("ident", [P, P], F32, isOutput=False)
    outp = nc.declare_dram_parameter("out", [P, DC, NLOC], F32, isOutput=True)
    combT_dram = nc.dram_tensor("combT_dram", [UNITS, nt128, P], BF)

    with tile.TileContext(nc) as tc:
      for _rep in range(repeat):
        with ExitStack() as ctx:
            # long-lived tiles
            const_pool = ctx.enter_context(tc.tile_pool(name="const", bufs=1))
            xtb_sb = const_pool.tile([P, DC, NLOC], BF)
            nc.sync.dma_start(xtb_sb[:], xtb[:, :, :])
            acc_sb = const_pool.tile([P, DC, NLOC], F32)

            # ---------------- Phase A: routing ----------------
            with ExitStack() as actx:
                apool = actx.enter_context(tc.tile_pool(name="routeA", bufs=1))
                rpool = actx.enter_context(tc.tile_pool(name="routeR", bufs=2))
                apsum = actx.enter_context(
                    tc.tile_pool(name="routeP", bufs=2, space="PSUM"))

                comb_sb = apool.tile([P, nt128, UNITS], F32)
                combT_sb = apool.tile([UNITS, nt128, P], BF)
                xt_sb = apool.tile([P, DC, NLOC], F32)
                nc.sync.dma_start(xt_sb[:], xt[:, :, :])
                gt_sb = apool.tile([P, DC, E], F32)
                nc.sync.dma_start(gt_sb[:], gt[:, :, :])
                id_sb = apool.tile([P, P], F32)
                nc.sync.dma_start(id_sb[:], ident[:, :])

                # shared-expert combine weight is 1
                nc.vector.memset(comb_sb[:, :, E], 1.0)

                for t in range(nt128):
                    ps_l = apsum.tile([P, E], F32, tag="psl")
                    for c in range(DC):
                        nc.tensor.matmul(
                            ps_l[:],
                            xt_sb[:, c, bass.ts(t, P)],
                            gt_sb[:, c, :],
                            start=(c == 0), stop=(c == DC - 1),
                        )
                    lt = rpool.tile([P, E], F32, tag="lt")
                    nc.scalar.copy(lt[:], ps_l[:])
                    m1 = rpool.tile([P, 1], F32, tag="m1")
                    nc.vector.reduce_max(m1[:], lt[:], axis=AX.X)
                    eq = rpool.tile([P, E], F32, tag="eq")
                    nc.vector.tensor_scalar(
                        eq[:], lt[:], m1[:], None, op0=AluOpType.is_equal)
                    l2 = rpool.tile([P, E], F32, tag="l2")
                    nc.vector.scalar_tensor_tensor(
                        l2[:], eq[:], -1e30, lt[:],
                        op0=AluOpType.mult, op1=AluOpType.add)
                    m2 = rpool.tile([P, 1], F32, tag="m2")
                    nc.vector.reduce_max(m2[:], l2[:], axis=AX.X)
                    nm1 = rpool.tile([P, 1], F32, tag="nm1")
                    nc.vector.tensor_scalar_mul(nm1[:], m1[:], -1.0)
                    ex = rpool.tile([P, E], F32, tag="ex")
                    nc.scalar.activation(ex[:], lt[:], AF.Exp, bias=nm1[:], scale=1.0)
                    mk = rpool.tile([P, E], F32, tag="mk")
                    nc.vector.tensor_scalar(
                        mk[:], lt[:], m2[:], None, op0=AluOpType.is_ge)
                    we = rpool.tile([P, E], F32, tag="we")
                    nc.vector.tensor_tensor(we[:], ex[:], mk[:], op=AluOpType.mult)
                    s = rpool.tile([P, 1], F32, tag="s")
                    nc.vector.reduce_sum(s[:], we[:], axis=AX.X)
                    rs = rpool.tile([P, 1], F32, tag="rs")
                    nc.vector.reciprocal(rs[:], s[:])
                    nc.vector.tensor_scalar(
                        comb_sb[:, t, 0:E], we[:], rs[:], None, op0=AluOpType.mult)

                    # comb tile [128, UNITS] -> combT [UNITS, 128]
                    ps_t = apsum.tile([P, P], F32, tag="pst")
                    nc.tensor.transpose(
                        ps_t[0:UNITS, :], comb_sb[:, t, :], id_sb[:])
                    nc.scalar.copy(combT_sb[:, t, :], ps_t[0:UNITS, :])
                nc.sync.dma_start(combT_dram[:, :, :], combT_sb[:])

            # ---------------- Phase B: experts ----------------
            with ExitStack() as bctx:
                wpool = bctx.enter_context(tc.tile_pool(name="wpool", bufs=1))
                cpool = bctx.enter_context(tc.tile_pool(name="cpool", bufs=1))
                hpool = bctx.enter_context(tc.tile_pool(name="hpool", bufs=2))
                spool = bctx.enter_context(tc.tile_pool(name="spool", bufs=2))
                gpsum = bctx.enter_context(
                    tc.tile_pool(name="gpsum", bufs=2, space="PSUM"))
                upsum = bctx.enter_context(
                    tc.tile_pool(name="upsum", bufs=2, space="PSUM"))
                ypsum = bctx.enter_context(
                    tc.tile_pool(name="ypsum", bufs=1, space="PSUM"))

                for u in range(UNITS):
                    wg_sb = wpool.tile([P, FC, DC, P], BF, tag="wg")
                    wu_sb = wpool.tile([P, FC, DC, P], BF, tag="wu")
                    wd_sb = wpool.tile([P, FC, D], BF, tag="wd")
                    for fc in range(FC):
                        nc.sync.dma_start(wg_sb[:, fc], wg[u, fc])
                        nc.sync.dma_start(wu_sb[:, fc], wu[u, fc])
                        nc.sync.dma_start(wd_sb[:, fc], wd[u, fc])

                    cb_u = cpool.tile([P, NLOC], BF, tag="cb")
                    nc.sync.dma_start(
                        cb_u[:],
                        combT_dram[u:u + 1].partition_broadcast(P).opt())

                    for tt in range(ntt):
                        hs_sb = hpool.tile([P, FC, TT], BF, tag="hs")
                        ps_y = ypsum.tile([P, DC * TT], F32, tag="py")
                        for fc in range(FC):
                            ps_g = gpsum.tile([P, TT], F32, tag="pg")
                            ps_u = upsum.tile([P, TT], F32, tag="pu")
                            for c in range(DC):
                                nc.tensor.matmul(
                                    ps_g[:],
                                    wg_sb[:, fc, c, :],
                                    xtb_sb[:, c, bass.ts(tt, TT)],
                                    start=(c == 0), stop=(c == DC - 1),
                                )
                            for c in range(DC):
                                nc.tensor.matmul(
                                    ps_u[:],
                                    wu_sb[:, fc, c, :],
                                    xtb_sb[:, c, bass.ts(tt, TT)],
                                    start=(c == 0), stop=(c == DC - 1),
                                )
                            sg = spool.tile([P, TT], F32, tag="sg")
                            nc.scalar.activation(sg[:], ps_g[:], AF.Silu)
                            h = spool.tile([P, TT], F32, tag="h")
                            nc.vector.tensor_tensor(
                                h[:], sg[:], ps_u[:], op=AluOpType.mult)
                            nc.vector.tensor_tensor(
                                hs_sb[:, fc, :], h[:], cb_u[:, bass.ts(tt, TT)],
                                op=AluOpType.mult)
                        for dcc in range(DC):
                            for fc in range(FC):
                                nc.tensor.matmul(
                                    ps_y[:, bass.ts(dcc, TT)],
                                    wd_sb[:, fc, bass.ts(dcc, P)],
                                    hs_sb[:, fc, :],
                                    start=(fc == 0), stop=(fc == FC - 1),
                                )
                        ps_y_v = ps_y[:].rearrange("p (c t) -> p c t", c=DC)
                        if u == 0:
                            nc.vector.tensor_copy(
                                acc_sb[:, :, bass.ts(tt, TT)], ps_y_v)
                        else:
                            nc.vector.tensor_tensor(
                                acc_sb[:, :, bass.ts(tt, TT)],
                                acc_sb[:, :, bass.ts(tt, TT)],
                                ps_y_v, op=AluOpType.add)

            nc.sync.dma_start(outp[:, :, :], acc_sb[:])
    if split_waits:
        _split_multi_waits(nc)
    return nc


def build_moe_v2(DC=8, FC=12, E=8, NLOC=2048, split_waits=True, repeat=1):
    """Dense v2: down-projection uses hs as the stationary operand with the
    full model dim as the moving axis (N=512 matmuls, half the instruction
    count of v1's N=256 form), output lands token-major, and the combine
    weight is applied in one fused multiply-add per unit on the DVE."""
    UNITS = E + 1
    D = DC * P
    nt128 = NLOC // P

    nc = bass.Bass(target_bir_lowering=False)
    xt = nc.declare_dram_parameter("xt", [P, DC, NLOC], F32, isOutput=False)
    xtb = nc.declare_dram_parameter("xtb", [P, DC, NLOC], BF, isOutput=False)
    gt = nc.declare_dram_parameter("gt", [P, DC, E], F32, isOutput=False)
    wg = nc.declare_dram_parameter("wg", [UNITS, FC, P, DC, P], BF, isOutput=False)
    wu = nc.declare_dram_parameter("wu", [UNITS, FC, P, DC, P], BF, isOutput=False)
    wd = nc.declare_dram_parameter("wd", [UNITS, FC, P, D], BF, isOutput=False)
    ident = nc.declare_dram_parameter("ident", [P, P], F32, isOutput=False)
    outp = nc.declare_dram_parameter("out", [NLOC, D], F32, isOutput=True)

    with tile.TileContext(nc) as tc:
      for _rep in range(repeat):
        with ExitStack() as ctx:
            const_pool = ctx.enter_context(tc.tile_pool(name="const", bufs=1))
            xtb_sb = const_pool.tile([P, DC, NLOC], BF)
            nc.sync.dma_start(xtb_sb[:], xtb[:, :, :])
            acc_sb = const_pool.tile([P, nt128, D], F32)
            comb_sb = const_pool.tile([P, nt128, UNITS], F32)
            nc.vector.memset(comb_sb[:, :, E], 1.0)

            # ---------------- Phase A: routing ----------------
            with ExitStack() as actx:
                apool = actx.enter_context(tc.tile_pool(name="routeA", bufs=1))
                rpool = actx.enter_context(tc.tile_pool(name="routeR", bufs=2))
                apsum = actx.enter_context(
                    tc.tile_pool(name="routeP", bufs=2, space="PSUM"))

                xt_sb = apool.tile([P, DC, NLOC], F32)
                nc.sync.dma_start(xt_sb[:], xt[:, :, :])
                gt_sb = apool.tile([P, DC, E], F32)
                nc.sync.dma_start(gt_sb[:], gt[:, :, :])
                id_sb = apool.tile([P, P], F32)
                nc.sync.dma_start(id_sb[:], ident[:, :])

                for t in range(nt128):
                    ps_l = apsum.tile([P, E], F32, tag="psl")
                    for c in range(DC):
                        nc.tensor.matmul(
                            ps_l[:],
                            xt_sb[:, c, bass.ts(t, P)],
                            gt_sb[:, c, :],
                            start=(c == 0), stop=(c == DC - 1),
                        )
                    lt = rpool.tile([P, E], F32, tag="lt")
                    nc.scalar.copy(lt[:], ps_l[:])
                    m1 = rpool.tile([P, 1], F32, tag="m1")
                    nc.vector.reduce_max(m1[:], lt[:], axis=AX.X)
                    eq = rpool.tile([P, E], F32, tag="eq")
                    nc.vector.tensor_scalar(
                        eq[:], lt[:], m1[:], None, op0=AluOpType.is_equal)
                    l2 = rpool.tile([P, E], F32, tag="l2")
                    nc.vector.scalar_tensor_tensor(
                        l2[:], eq[:], -1e30, lt[:],
                        op0=AluOpType.mult, op1=AluOpType.add)
                    m2 = rpool.tile([P, 1], F32, tag="m2")
                    nc.vector.reduce_max(m2[:], l2[:], axis=AX.X)
                    nm1 = rpool.tile([P, 1], F32, tag="nm1")
                    nc.vector.tensor_scalar_mul(nm1[:], m1[:], -1.0)
                    ex = rpool.tile([P, E], F32, tag="ex")
                    nc.scalar.activation(ex[:], lt[:], AF.Exp, bias=nm1[:], scale=1.0)
                    mk = rpool.tile([P, E], F32, tag="mk")
                    nc.vector.tensor_scalar(
                        mk[:], lt[:], m2[:], None, op0=AluOpType.is_ge)
                    we = rpool.tile([P, E], F32, tag="we")
                    nc.vector.tensor_tensor(we[:], ex[:], mk[:], op=AluOpType.mult)
                    s = rpool.tile([P, 1], F32, tag="s")
                    nc.vector.reduce_sum(s[:], we[:], axis=AX.X)
                    rs = rpool.tile([P, 1], F32, tag="rs")
                    nc.vector.reciprocal(rs[:], s[:])
                    nc.vector.tensor_scalar(
                        comb_sb[:, t, 0:E], we[:], rs[:], None, op0=AluOpType.mult)

            # ---------------- Phase B: experts ----------------
            with ExitStack() as bctx:
                wpool = bctx.enter_context(tc.tile_pool(name="wpool", bufs=1))
                hpool = bctx.enter_context(tc.tile_pool(name="hpool", bufs=3))
                spool = bctx.enter_context(tc.tile_pool(name="spool", bufs=2))
                gpsum = bctx.enter_context(
                    tc.tile_pool(name="gpsum", bufs=2, space="PSUM"))
                upsum = bctx.enter_context(
                    tc.tile_pool(name="upsum", bufs=2, space="PSUM"))
                ypsum = bctx.enter_context(
                    tc.tile_pool(name="ypsum", bufs=2, space="PSUM"))

                dw = min(512, D)
                for u in range(UNITS):
                    wg_sb = wpool.tile([P, FC, DC, P], BF, tag="wg")
                    wu_sb = wpool.tile([P, FC, DC, P], BF, tag="wu")
                    wd_sb = wpool.tile([P, FC, D], BF, tag="wd")
                    for fc in range(FC):
                        nc.sync.dma_start(wg_sb[:, fc], wg[u, fc])
                        nc.scalar.dma_start(wu_sb[:, fc], wu[u, fc])
                        nc.gpsimd.dma_start(wd_sb[:, fc], wd[u, fc])

                    # g/u at the widest moving dim the psum budget allows;
                    # hs stays 256-wide for the down stage
                    TW = 512 if NLOC % 512 == 0 else 256
                    NH = TW // 256
                    for tt in range(NLOC // TW):
                        ps_g = gpsum.tile([P, TW], F32, tag="pg")
                        ps_u = upsum.tile([P, TW], F32, tag="pu")
                        hs_tiles = [
                            hpool.tile([P, FC, 256], BF, tag="hs",
                                       name=f"hs_{u}_{tt}_{h}")
                            for h in range(NH)]
                        for fc in range(FC):
                            for c in range(DC):
                                nc.tensor.matmul(
                                    ps_g[:], wg_sb[:, fc, c, :],
                                    xtb_sb[:, c, bass.ts(tt, TW)],
                                    start=(c == 0), stop=(c == DC - 1))
                            for c in range(DC):
                                nc.tensor.matmul(
                                    ps_u[:], wu_sb[:, fc, c, :],
                                    xtb_sb[:, c, bass.ts(tt, TW)],
                                    start=(c == 0), stop=(c == DC - 1))
                            sg_t = spool.tile([P, TW], F32, tag="sg")
                            nc.scalar.activation(sg_t[:], ps_g[:], AF.Silu)
                            for h in range(NH):
                                nc.vector.tensor_tensor(
                                    hs_tiles[h][:, fc, :],
                                    sg_t[:, h * 256:(h + 1) * 256],
                                    ps_u[:, h * 256:(h + 1) * 256],
                                    op=AluOpType.mult)
                        for h in range(NH):
                            for sub in range(2):
                                t128 = (tt * NH + h) * 2 + sub
                                yp = ypsum.tile([P, D], F32, tag="yp")
                                for half in range(D // dw):
                                    for fc in range(FC):
                                        nc.tensor.matmul(
                                            yp[:, half * dw:(half + 1) * dw],
                                            hs_tiles[h][:, fc,
                                                        sub * P:(sub + 1) * P],
                                            wd_sb[:, fc,
                                                  half * dw:(half + 1) * dw],
                                            start=(fc == 0),
                                            stop=(fc == FC - 1))
                                if u == 0:
                                    nc.vector.tensor_scalar(
                                        acc_sb[:, t128, :], yp[:],
                                        comb_sb[:, t128, u:u + 1], None,
                                        op0=AluOpType.mult)
                                else:
                                    nc.vector.scalar_tensor_tensor(
                                        acc_sb[:, t128, :], yp[:],
                                        comb_sb[:, t128, u:u + 1],
                                        acc_sb[:, t128, :],
                                        op0=AluOpType.mult, op1=AluOpType.add)

            nc.sync.dma_start(
                outp[:, :].rearrange("(t p) d -> p t d", p=P), acc_sb[:])
    if split_waits:
        _split_multi_waits(nc)
    return nc


def build_moe_sparse(DC=8, FC=12, E=8, NLOC=2048, CAP=640, split_waits=True,
                     repeat=1, debug_idx=False):
    """Sparse (top-2 gathered) variant.

    Routing runs as in the dense kernel; per expert, selected token indices
    are compacted on device (sparse_gather), token activations are gathered
    transposed straight into the d-chunked matmul layout (dma_gather), the
    expert SwiGLU runs only on CAP capacity slots, outputs are scaled by the
    gathered combine weight and scatter-added into a token-major DRAM
    accumulator primed by the shared expert. Pad slots point at a zeroed
    dummy token row so every op stays static-shape.
    """
    from concourse import library_config

    UNITS = E + 1
    D = DC * P
    nt128 = NLOC // P
    F16 = NLOC // 16
    CW = CAP // 16
    NST = CAP // P
    DUMMY = NLOC  # index of the zeroed dummy row
    ST_LIST = []
    s0 = 0
    while s0 < CAP:
        w = min(256, CAP - s0)
        ST_LIST.append((s0, w))
        s0 += w

    nc = bass.Bass(target_bir_lowering=False)
    xt = nc.declare_dram_parameter("xt", [P, DC, NLOC], F32, isOutput=False)
    xtb = nc.declare_dram_parameter("xtb", [P, DC, NLOC], BF, isOutput=False)
    xb = nc.declare_dram_parameter("xb", [NLOC + 16, D], BF, isOutput=False)
    gt = nc.declare_dram_parameter("gt", [P, DC, E], F32, isOutput=False)
    wg = nc.declare_dram_parameter("wg", [UNITS, FC, P, DC, P], BF, isOutput=False)
    wu = nc.declare_dram_parameter("wu", [UNITS, FC, P, DC, P], BF, isOutput=False)
    wd = nc.declare_dram_parameter("wd", [UNITS, FC, P, D], BF, isOutput=False)
    ident = nc.declare_dram_parameter("ident", [P, P], F32, isOutput=False)
    iota16 = nc.declare_dram_parameter("iota16", [16, F16], F32, isOutput=False)
    outp = nc.declare_dram_parameter("out", [NLOC, D], F32, isOutput=True)
    if debug_idx:
        idxdbg = nc.declare_dram_parameter(
            "idxdbg", [P, E, CW], mybir.dt.int16, isOutput=True)
    combT_dram = nc.dram_tensor("combT_dram", [E, nt128, P], BF)
    combR_dram = nc.dram_tensor("combR_dram", [E, NLOC + 16, 64], F32)
    acc_dram = nc.dram_tensor("acc_dram", [NLOC + 16, D], F32)
    nf_dram = nc.dram_tensor("nf_dram", [1, E], F32)

    with tile.TileContext(nc) as tc:
      for _rep in range(repeat):
        with ExitStack() as ctx:
            const_pool = ctx.enter_context(tc.tile_pool(name="const", bufs=1))
            xtb_sb = const_pool.tile([P, DC, NLOC], BF)
            nc.sync.dma_start(xtb_sb[:], xtb[:, :, :])
            idx_sb = const_pool.tile([P, E, CW], mybir.dt.int16)

            # ---------------- Phase A: routing + index build ----------------
            with ExitStack() as actx:
                apool = actx.enter_context(tc.tile_pool(name="routeA", bufs=1))
                rpool = actx.enter_context(tc.tile_pool(name="routeR", bufs=2))
                apsum = actx.enter_context(
                    tc.tile_pool(name="routeP", bufs=2, space="PSUM"))

                comb_sb = apool.tile([P, nt128, E], F32)
                combT_sb = apool.tile([E, nt128, P], BF)
                xt_sb = apool.tile([P, DC, NLOC], F32)
                nc.sync.dma_start(xt_sb[:], xt[:, :, :])
                gt_sb = apool.tile([P, DC, E], F32)
                nc.sync.dma_start(gt_sb[:], gt[:, :, :])
                id_sb = apool.tile([P, P], F32)
                nc.sync.dma_start(id_sb[:], ident[:, :])
                iota_sb = apool.tile([16, F16], F32)
                nc.sync.dma_start(iota_sb[:], iota16[:, :])
                neg1_sb = apool.tile([16, F16], F32)
                nc.vector.memset(neg1_sb[:], -1.0)
                dummy_sb = apool.tile([16, F16], F32)
                nc.vector.memset(dummy_sb[:], float(DUMMY))
                zz_sb = apool.tile([16, 64], F32)
                nc.vector.memset(zz_sb[:], 0.0)

                for t in range(nt128):
                    ps_l = apsum.tile([P, E], F32, tag="psl")
                    for c in range(DC):
                        nc.tensor.matmul(
                            ps_l[:],
                            xt_sb[:, c, bass.ts(t, P)],
                            gt_sb[:, c, :],
                            start=(c == 0), stop=(c == DC - 1),
                        )
                    lt = rpool.tile([P, E], F32, tag="lt")
                    nc.scalar.copy(lt[:], ps_l[:])
                    m1 = rpool.tile([P, 1], F32, tag="m1")
                    nc.vector.reduce_max(m1[:], lt[:], axis=AX.X)
                    eq = rpool.tile([P, E], F32, tag="eq")
                    nc.vector.tensor_scalar(
                        eq[:], lt[:], m1[:], None, op0=AluOpType.is_equal)
                    l2 = rpool.tile([P, E], F32, tag="l2")
                    nc.vector.scalar_tensor_tensor(
                        l2[:], eq[:], -1e30, lt[:],
                        op0=AluOpType.mult, op1=AluOpType.add)
                    m2 = rpool.tile([P, 1], F32, tag="m2")
                    nc.vector.reduce_max(m2[:], l2[:], axis=AX.X)
                    nm1 = rpool.tile([P, 1], F32, tag="nm1")
                    nc.vector.tensor_scalar_mul(nm1[:], m1[:], -1.0)
                    ex = rpool.tile([P, E], F32, tag="ex")
                    nc.scalar.activation(ex[:], lt[:], AF.Exp, bias=nm1[:], scale=1.0)
                    mk = rpool.tile([P, E], F32, tag="mk")
                    nc.vector.tensor_scalar(
                        mk[:], lt[:], m2[:], None, op0=AluOpType.is_ge)
                    we = rpool.tile([P, E], F32, tag="we")
                    nc.vector.tensor_tensor(we[:], ex[:], mk[:], op=AluOpType.mult)
                    s = rpool.tile([P, 1], F32, tag="s")
                    nc.vector.reduce_sum(s[:], we[:], axis=AX.X)
                    rs = rpool.tile([P, 1], F32, tag="rs")
                    nc.vector.reciprocal(rs[:], s[:])
                    nc.vector.tensor_scalar(
                        comb_sb[:, t, :], we[:], rs[:], None, op0=AluOpType.mult)

                    ps_t = apsum.tile([P, P], F32, tag="pst")
                    nc.tensor.transpose(
                        ps_t[0:E, :], comb_sb[:, t, :], id_sb[:])
                    nc.scalar.copy(combT_sb[:, t, :], ps_t[0:E, :])
                nc.sync.dma_start(combT_dram[:, :, :], combT_sb[:])

                # combine-weight gather table (64-wide replicated, f32),
                # plus zeroed dummy rows
                for e in range(E):
                    crv = rpool.tile([P, nt128, 64], F32, tag="crv")
                    nc.vector.tensor_copy(
                        crv[:], comb_sb[:, :, e:e + 1].to_broadcast(
                            (P, nt128, 64)))
                    nc.sync.dma_start(
                        combR_dram[e, 0:NLOC].rearrange(
                            "(t p) r -> p t r", p=P),
                        crv[:])
                    nc.sync.dma_start(
                        combR_dram[e, NLOC:NLOC + 16], zz_sb[:])
                # zero acc dummy rows (scatter-add RMW reads them)
                zd_sb = apool.tile([16, D], F32)
                nc.vector.memset(zd_sb[:], 0.0)
                nc.sync.dma_start(acc_dram[NLOC:NLOC + 16, :], zd_sb[:])

                # per-expert index compaction. The HW ucode writes garbage
                # past the num_found prefix, so every slot is additionally
                # masked by its rank against the device-side num_found
                # (broadcast to all 16 partitions via a DRAM roundtrip).
                lib_sg = nc.gpsimd.load_library(library_config.sparse_gather)
                sg_insts = []
                idxf8 = apool.tile([16, E, F16], F32)
                nf8 = apool.tile([1, E], mybir.dt.uint32)
                for e in range(E):
                    mv = rpool.tile([16, F16], BF, tag="mv")
                    nc.sync.dma_start(
                        mv[:],
                        combT_dram[e].rearrange("a b -> (a b)").rearrange(
                            "(f q) -> q f", q=16))
                    msk = rpool.tile([16, F16], mybir.dt.uint8, tag="msk")
                    nc.vector.tensor_scalar(
                        msk[:], mv[:], 0.0, None, op0=AluOpType.is_gt)
                    tokneg = rpool.tile([16, F16], F32, tag="tokneg")
                    nc.vector.select(tokneg[:], msk[:], iota_sb[:], neg1_sb[:])
                    sg = nc.gpsimd.sparse_gather(
                        idxf8[:, e, :], tokneg[:], num_found=nf8[:, e:e + 1])
                    tile.add_dep_helper(sg.ins, lib_sg.ins,
                                        reason="sparse_gather lib")
                    sg_insts.append(sg)

                nff = rpool.tile([1, E], F32, tag="nff")
                nc.vector.tensor_copy(nff[:], nf8[:])
                nc.sync.dma_start(nf_dram[:, :], nff[:])
                nfb = rpool.tile([16, E], F32, tag="nfb")
                nc.sync.dma_start(
                    nfb[:], nf_dram[0:1, :].partition_broadcast(16).opt())

                for e in range(E):
                    rk = rpool.tile([16, F16], mybir.dt.uint8, tag="rk")
                    nc.vector.tensor_scalar(
                        rk[:], iota_sb[:], nfb[:, e:e + 1], None,
                        op0=AluOpType.is_lt)
                    ge0 = rpool.tile([16, F16], mybir.dt.uint8, tag="ge0")
                    nc.vector.tensor_scalar(
                        ge0[:], idxf8[:, e, :], 0.0, None, op0=AluOpType.is_ge)
                    ltn = rpool.tile([16, F16], mybir.dt.uint8, tag="ltn")
                    nc.vector.tensor_scalar(
                        ltn[:], idxf8[:, e, :], float(NLOC), None,
                        op0=AluOpType.is_lt)
                    nc.vector.tensor_tensor(
                        ge0[:], ge0[:], ltn[:], op=AluOpType.bitwise_and)
                    nc.vector.tensor_tensor(
                        ge0[:], ge0[:], rk[:], op=AluOpType.bitwise_and)
                    idcl = rpool.tile([16, F16], F32, tag="idcl")
                    nc.vector.select(
                        idcl[:], ge0[:], idxf8[:, e, :], dummy_sb[:])
                    idci = rpool.tile([16, F16], mybir.dt.int16, tag="idci")
                    nc.vector.tensor_copy(idci[:], idcl[:])
                    for g in range(8):
                        nc.sync.dma_start(
                            idx_sb[16 * g:16 * (g + 1), e, :], idci[:, 0:CW])

                lib_mlp = nc.gpsimd.load_library(library_config.mlp)
                for sg in sg_insts:
                    tile.add_dep_helper(lib_mlp.ins, sg.ins,
                                        reason="mlp lib after sparse_gather")

            # ---------------- Phase B: experts ----------------
            with ExitStack() as bctx:
                wpool = bctx.enter_context(tc.tile_pool(name="wpool", bufs=1))
                xgpool = bctx.enter_context(tc.tile_pool(name="xgpool", bufs=2))
                cgpool = bctx.enter_context(tc.tile_pool(name="cgpool", bufs=2))
                hpool = bctx.enter_context(tc.tile_pool(name="hpool", bufs=2))
                spool = bctx.enter_context(tc.tile_pool(name="spool", bufs=2))
                ypool = bctx.enter_context(tc.tile_pool(name="ypool", bufs=1))
                gpsum = bctx.enter_context(
                    tc.tile_pool(name="gpsum", bufs=2, space="PSUM"))
                upsum = bctx.enter_context(
                    tc.tile_pool(name="upsum", bufs=2, space="PSUM"))
                ypsum = bctx.enter_context(
                    tc.tile_pool(name="ypsum", bufs=2, space="PSUM"))

                def load_unit_weights(u):
                    wg_sb = wpool.tile([P, FC, DC, P], BF, tag="wg")
                    wu_sb = wpool.tile([P, FC, DC, P], BF, tag="wu")
                    wd_sb = wpool.tile([P, FC, D], BF, tag="wd")
                    for fc in range(FC):
                        nc.sync.dma_start(wg_sb[:, fc], wg[u, fc])
                        nc.sync.dma_start(wu_sb[:, fc], wu[u, fc])
                        nc.sync.dma_start(wd_sb[:, fc], wd[u, fc])
                    return wg_sb, wu_sb, wd_sb

                def gu_sweep(wg_sb, wu_sb, rhs_fn, width):
                    # one token/slot tile: returns hs [P, FC, width] bf16
                    hs_sb = hpool.tile([P, FC, 256], BF, tag="hs")
                    for fc in range(FC):
                        ps_g = gpsum.tile([P, 256], F32, tag="pg")
                        ps_u = upsum.tile([P, 256], F32, tag="pu")
                        for c in range(DC):
                            nc.tensor.matmul(
                                ps_g[:, 0:width], wg_sb[:, fc, c, :], rhs_fn(c),
                                start=(c == 0), stop=(c == DC - 1))
                        for c in range(DC):
                            nc.tensor.matmul(
                                ps_u[:, 0:width], wu_sb[:, fc, c, :], rhs_fn(c),
                                start=(c == 0), stop=(c == DC - 1))
                        sg_t = spool.tile([P, 256], F32, tag="sg")
                        nc.scalar.activation(
                            sg_t[:, 0:width], ps_g[:, 0:width], AF.Silu)
                        nc.vector.tensor_tensor(
                            hs_sb[:, fc, 0:width], sg_t[:, 0:width],
                            ps_u[:, 0:width], op=AluOpType.mult)
                    return hs_sb

                def down_sub(hs_sb, wd_sb, sub):
                    # one 128-slot subtile -> psum [P, D]
                    yp = ypsum.tile([P, D], F32, tag="yp")
                    dw = min(512, D)
                    for half in range(D // dw):
                        for fc in range(FC):
                            nc.tensor.matmul(
                                yp[:, half * dw:(half + 1) * dw],
                                hs_sb[:, fc, sub * P:(sub + 1) * P],
                                wd_sb[:, fc, half * dw:(half + 1) * dw],
                                start=(fc == 0), stop=(fc == FC - 1))
                    return yp

                # shared expert: dense over all tokens, direct row writes
                wg_sb, wu_sb, wd_sb = load_unit_weights(E)
                for tt in range(NLOC // 256):
                    hs_sb = gu_sweep(
                        wg_sb, wu_sb,
                        lambda c: xtb_sb[:, c, bass.ts(tt, 256)], 256)
                    for sub in range(2):
                        yp = down_sub(hs_sb, wd_sb, sub)
                        ysh = spool.tile([P, D], F32, tag="ysh")
                        nc.scalar.copy(ysh[:], yp[:])
                        r0 = (tt * 2 + sub) * P
                        nc.sync.dma_start(acc_dram[r0:r0 + P, :], ysh[:])

                # routed experts: gathered slots, comb-scaled scatter-add
                for e in range(E):
                    wg_sb, wu_sb, wd_sb = load_unit_weights(e)
                    xg_sb = xgpool.tile([P, DC, CAP], BF, tag="xg")
                    g1 = nc.gpsimd.dma_gather(
                        xg_sb[:], xb[:, :], idx_sb[:, e, :],
                        num_idxs=CAP, num_idxs_reg=CAP, elem_size=D,
                        transpose=True)
                    tile.add_dep_helper(g1.ins, lib_mlp.ins, reason="mlp lib")
                    cg_sb = cgpool.tile([P, NST, 64], F32, tag="cg")
                    g2 = nc.gpsimd.dma_gather(
                        cg_sb[:], combR_dram[e], idx_sb[:, e, :],
                        num_idxs=CAP, num_idxs_reg=CAP, elem_size=64,
                        transpose=False)
                    tile.add_dep_helper(g2.ins, lib_mlp.ins, reason="mlp lib")

                    ysc = ypool.tile([P, NST, D], F32, tag="ysc")
                    for (s0, sw) in ST_LIST:
                        hs_sb = gu_sweep(
                            wg_sb, wu_sb,
                            lambda c: xg_sb[:, c, s0:s0 + sw], sw)
                        for sub in range(sw // P):
                            gsub = s0 // P + sub
                            yp = down_sub(hs_sb, wd_sb, sub)
                            nc.vector.tensor_scalar(
                                ysc[:, gsub, :], yp[:], cg_sb[:, gsub, 0:1],
                                None, op0=AluOpType.mult)
                    sc = nc.gpsimd.dma_scatter_add(
                        acc_dram[:, :], ysc[:], idx_sb[:, e, :],
                        num_idxs=CAP, num_idxs_reg=CAP, elem_size=D)
                    tile.add_dep_helper(sc.ins, lib_mlp.ins, reason="mlp lib")

                nc.sync.dma_start(outp[:, :], acc_dram[0:NLOC, :])
                if debug_idx:
                    nc.sync.dma_start(idxdbg[:, :, :], idx_sb[:])
    from concourse.library_overlay import lower_extended_insts
    lower_extended_insts(nc)
    if split_waits:
        _split_multi_waits(nc)
    return nc


# ---------------------------------------------------------------------------
# Host side


def _prep_weight_gu(w, DC, FC):
    # w [HALF, D] -> [FC, 128, DC, 128]: out[fc, p, c, f] = w[fc*128+f, c*128+p]
    D = DC * P
    HALF = FC * P
    wt = w.T.reshape(DC, P, FC, P).transpose(2, 1, 0, 3)
    return np.ascontiguousarray(wt.astype(bf16))


def _prep_weight_d(w, DC, FC):
    # w [D, HALF] -> [FC, 128, D]: out[fc, p, d] = w[d, fc*128+p]
    wt = w.T.reshape(FC, P, DC * P)
    return np.ascontiguousarray(wt.astype(bf16))


_BUILT = {}

USE_SPARSE = True
USE_V2 = False


def _get_built(key, **kw):
    if key not in _BUILT:
        if USE_SPARSE:
            _BUILT[key] = build_moe_sparse(**kw)
        elif USE_V2:
            _BUILT[key] = build_moe_v2(**kw)
        else:
            _BUILT[key] = build_moe(**kw)
    return _BUILT[key]


def prepare(x, gate_w, w_up, w_down, sg_gate, sg_up, sg_down):
    """Build (nc, in_maps, meta) for the 8-core SPMD launch."""
    B, T, D = x.shape
    E = gate_w.shape[0]
    FFN = w_up.shape[1]
    HALF = FFN // 2
    DC = D // P
    FC = HALF // P
    N = B * T
    NCORES = 8
    NLOC = N // NCORES

    nc = _get_built((DC, FC, E, NLOC), DC=DC, FC=FC, E=E, NLOC=NLOC)

    UNITS = E + 1
    wg_all = np.empty((UNITS, FC, P, DC, P), dtype=bf16)
    wu_all = np.empty((UNITS, FC, P, DC, P), dtype=bf16)
    wd_all = np.empty((UNITS, FC, P, D), dtype=bf16)
    for u in range(E):
        wg_all[u] = _prep_weight_gu(w_up[u, :HALF], DC, FC)
        wu_all[u] = _prep_weight_gu(w_up[u, HALF:], DC, FC)
        wd_all[u] = _prep_weight_d(w_down[u], DC, FC)
    wg_all[E] = _prep_weight_gu(sg_gate, DC, FC)
    wu_all[E] = _prep_weight_gu(sg_up, DC, FC)
    wd_all[E] = _prep_weight_d(sg_down, DC, FC)

    gt = np.ascontiguousarray(
        gate_w.T.reshape(DC, P, E).transpose(1, 0, 2).astype(np.float32))
    ident = np.eye(P, dtype=np.float32)
    F16 = NLOC // 16
    iota16 = np.ascontiguousarray(
        (np.arange(F16)[None, :] * 16 + np.arange(16)[:, None])
        .astype(np.float32))

    xf = np.ascontiguousarray(x.reshape(N, D))
    in_maps = []
    for ci in range(NCORES):
        xc = xf[ci * NLOC:(ci + 1) * NLOC]
        xt = np.ascontiguousarray(
            xc.T.reshape(DC, P, NLOC).transpose(1, 0, 2).astype(np.float32))
        xtb = xt.astype(bf16)
        m = {
            "xt": xt, "xtb": xtb, "gt": gt,
            "wg": wg_all, "wu": wu_all, "wd": wd_all,
            "ident": ident,
        }
        if USE_SPARSE:
            xbp = np.zeros((NLOC + 16, D), dtype=bf16)
            xbp[:NLOC] = xc.astype(bf16)
            m["xb"] = xbp
            m["iota16"] = iota16
        in_maps.append(m)

    return nc, in_maps, (B, T, D, NLOC, NCORES)


def postprocess(results, meta):
    B, T, D, NLOC, NCORES = meta
    outs = []
    for ci in range(NCORES):
        o = results[ci]["out"]
        if USE_SPARSE or USE_V2:
            outs.append(o.reshape(NLOC, D))
        else:
            DC = D // P
            outs.append(
                o.reshape(P, DC, NLOC).transpose(1, 0, 2).reshape(D, NLOC).T)
    return np.concatenate(outs, axis=0).reshape(B, T, D).astype(np.float32)


def kernel(x, gate_w, w_up, w_down, sg_gate, sg_up, sg_down):
    from concourse.bass_utils import run_bass_kernel_spmd

    nc, in_maps, meta = prepare(
        x, gate_w, w_up, w_down, sg_gate, sg_up, sg_down)
    r = run_bass_kernel_spmd(nc, in_maps, core_ids=list(range(meta[4])))
    return postprocess(r.results, meta)

